# revision 4
# baseline (speedup 1.0000x reference)
"""Trainium2 Bass kernel for nn_BL_36721970381090 (dense_mlp).

Math: the reference network collapses (einsum commutation + Kronecker
structure) to, per batch row b:
    z[d, u]   = sum_s fc2_w[u, s] * x[b, d, s]          (200 feats)
    item2[t,u]= sum_d W11[t, d] * z[d, u] + bias1[t, u] (600 feats)
    out[b, o] = sum_{t,u} W12[o, t] * fc4_w[0, u] * relu(item2[t, u]) + bias2[o, 0]

Strategy: pure data parallel over 8 NeuronCores (batch 131072 -> 8 x 16384),
all compute in bf16 moving / fp32 psum. Per 512-col block the PE runs just
14 matmul passes (vs 19 in the naive chunked form):
  stage1 (4 mm): x 4 chunks [100,512]; two chunks accumulate into one psum
     tile via block-diagonal stationaries, giving z in a u-major layout
     (row u*20 + q*10 + dl, i.e. d-contiguous per u within a chunk pair).
  z copy (2 engine ops) psum->sbuf bf16, then 10 SBUF->SBUF DMAs gather
     per-u tiles y_u [40, 512] (d = 0..40 contiguous).
  stage2 (5 mm): ps1_u [120,512] = W11T [40,120].T @ y_u  -- K=40 per-u,
     one matmul per u instead of (2 z-tiles x 5 m-chunks).
  relu+bias (5 engine ops) -> r_u [120,512] bf16.
  stage3 (5 mm): ps2 [3,512] += m2_u [120,3].T @ r_u.
bias2 folded on host after gather.
"""

import numpy as np
import ml_dtypes
from contextlib import ExitStack

import concourse.bass as bass
import concourse.bacc as bacc
import concourse.mybir as mybir
from concourse.bass import ds
from concourse.tile import TileContext
from concourse.bass_utils import run_bass_kernel_spmd

B, D1, D2 = 131072, 40, 10
T0, T1, O0 = 120, 5, 3
NCORES = 8
BC = B // NCORES          # 16384 batch per core
KF = D1 * D2              # 400 input features (d, s)
KC = 100                  # stage-1 K-chunk (4 chunks of 100 partitions)
NB = 512                  # matmul free-dim block (1 PSUM bank fp32)
NBD = 2048                # DMA block (4 x NB)

F32 = mybir.dt.float32
BF16 = mybir.dt.bfloat16
BF = ml_dtypes.bfloat16
RELU = mybir.ActivationFunctionType.Relu
COPY = mybir.ActivationFunctionType.Copy
ADD = mybir.AluOpType.add
MAX = mybir.AluOpType.max

_CACHE = {}


def _build_nc():
    nc = bacc.Bacc()
    xt = nc.dram_tensor("xt", (KF, BC), BF16, kind="ExternalInput")
    a0_h = nc.dram_tensor("a0", (KC, KC), BF16, kind="ExternalInput")
    a1_h = nc.dram_tensor("a1", (KC, KC), BF16, kind="ExternalInput")
    w11t_h = nc.dram_tensor("w11t", (D1, T0), BF16, kind="ExternalInput")
    m2_h = nc.dram_tensor("m2", (T0, T1 * O0), BF16, kind="ExternalInput")
    b1_h = nc.dram_tensor("b1", (T0, T1), F32, kind="ExternalInput")
    outT = nc.dram_tensor("outT", (O0, BC), F32, kind="ExternalOutput")

    with TileContext(nc) as tc, ExitStack() as ctx:
        consts = ctx.enter_context(tc.tile_pool(name="consts", bufs=1))
        a_sb = [consts.tile([KC, KC], BF16, tag=f"a{q}", name=f"a{q}") for q in range(2)]
        nc.sync.dma_start(a_sb[0][:, :], a0_h[:, :])
        nc.sync.dma_start(a_sb[1][:, :], a1_h[:, :])
        w11t_sb = consts.tile([D1, T0], BF16, tag="w11t")
        nc.sync.dma_start(w11t_sb[:, :], w11t_h[:, :])
        m2_sb = consts.tile([T0, T1 * O0], BF16, tag="m2")
        nc.sync.dma_start(m2_sb[:, :], m2_h[:, :])
        b1_sb = consts.tile([T0, T1], F32, tag="b1")
        nc.sync.dma_start(b1_sb[:, :], b1_h[:, :])

        xpool = ctx.enter_context(tc.tile_pool(name="xp", bufs=3))
        zpool = ctx.enter_context(tc.tile_pool(name="zp", bufs=2))
        ypool = ctx.enter_context(tc.tile_pool(name="yp", bufs=2))
        rpool = ctx.enter_context(tc.tile_pool(name="rp", bufs=2))
        opool = ctx.enter_context(tc.tile_pool(name="op", bufs=2))
        pzp = ctx.enter_context(tc.tile_pool(name="pz", bufs=1, space="PSUM"))
        ps1p = ctx.enter_context(tc.tile_pool(name="ps1", bufs=4, space="PSUM"))
        ps2p = ctx.enter_context(tc.tile_pool(name="ps2", bufs=2, space="PSUM"))

        for blk in range(BC // NBD):
            if blk == 0:
                x0 = [
                    [xpool.tile([KC, NB], BF16, tag=f"w{k}_{jj}", name=f"x0_{k}_{jj}")
                     for k in range(4)]
                    for jj in range(NBD // NB)
                ]
                for jj in range(NBD // NB):
                    for k in range(4):
                        nc.sync.dma_start(
                            x0[jj][k][:, :],
                            xt[ds(k * KC, KC), ds(jj * NB, NB)],
                        )
            else:
                xk = [xpool.tile([KC, NBD], BF16, tag=f"x{k}", name=f"xk{k}") for k in range(4)]
                for k in range(4):
                    nc.sync.dma_start(xk[k][:, :], xt[ds(k * KC, KC), ds(blk * NBD, NBD)])
            for jj in range(NBD // NB):
                def xs(k):
                    return (x0[jj][k][:, :] if blk == 0
                            else xk[k][:, ds(jj * NB, NB)])
                # stage 1: pair p accumulates chunks 2p, 2p+1 into one psum
                # tile with u-major rows (u*20 + q*10 + dl)
                ztiles = []
                for p in range(2):
                    pz = pzp.tile([KC, NB], F32, tag=f"pz{p}", name=f"pz{p}{jj}")
                    nc.tensor.matmul(pz[:, :], a_sb[0][:, :], xs(2 * p),
                                     start=True, stop=False)
                    nc.tensor.matmul(pz[:, :], a_sb[1][:, :], xs(2 * p + 1),
                                     start=False, stop=True)
                    z = zpool.tile([KC, NB], BF16, tag=f"z{p}", name=f"zt{p}{jj}")
                    if p == 0:
                        nc.scalar.activation(z[:, :], pz[:, :], COPY)
                    else:
                        nc.vector.tensor_copy(z[:, :], pz[:, :])
                    ztiles.append(z)
                # gather per-u tiles y_u [40, 512] (d contiguous 0..40)
                ytiles = []
                for u in range(T1):
                    y = ypool.tile([D1, NB], BF16, tag=f"y{u}", name=f"y{u}{jj}")
                    nc.sync.dma_start(y[ds(0, 20), :], ztiles[0][ds(20 * u, 20), :])
                    nc.sync.dma_start(y[ds(20, 20), :], ztiles[1][ds(20 * u, 20), :])
                    ytiles.append(y)
                # stage 2 per-u + relu + stage 3
                if jj == 0:
                    osb = opool.tile([O0, NBD], F32, tag="osb")
                ps2 = ps2p.tile([O0, NB], F32, tag="ps2", name=f"ps2{jj}")
                rtiles = []
                for u in range(T1):
                    pp = ps1p.tile([T0, NB], F32, tag="ps1", name=f"pp{u}{jj}")
                    nc.tensor.matmul(pp[:, :], w11t_sb[:, :], ytiles[u][:, :],
                                     start=True, stop=True)
                    r = rpool.tile([T0, NB], BF16, tag=f"r{u}", name=f"rt{u}{jj}")
                    if u < 3:
                        nc.scalar.activation(r[:, :], pp[:, :], RELU,
                                             bias=b1_sb[:, ds(u, 1)])
                    else:
                        nc.vector.tensor_scalar(r[:, :], pp[:, :],
                                                b1_sb[:, ds(u, 1)], 0.0,
                                                op0=ADD, op1=MAX)
                    rtiles.append(r)
                for u in range(T1):
                    nc.tensor.matmul(ps2[:, :], m2_sb[:, ds(u * O0, O0)],
                                     rtiles[u][:, :],
                                     start=(u == 0), stop=(u == T1 - 1))
                nc.vector.tensor_copy(osb[:, ds(jj * NB, NB)], ps2[:, :])
            nc.sync.dma_start(outT[:, ds(blk * NBD, NBD)], osb[:, :])
    nc.finalize()
    return nc


def _host_prep(W11, fc2_w, bias1, W12, fc4_w):
    # stage-1 stationaries: row rc = dl*10 + s; col m = u*20 + q*10 + dl
    a = np.zeros((2, KC, KC), np.float32)
    for q in range(2):
        for dl in range(10):
            for s in range(10):
                for u in range(T1):
                    a[q, dl * 10 + s, u * 20 + q * 10 + dl] = fc2_w[u, s]
    w11t = np.ascontiguousarray(W11.T)                      # [40, 120]
    m2 = np.empty((T0, T1 * O0), np.float32)                # [t, u*3+o]
    for u in range(T1):
        for o in range(O0):
            m2[:, u * O0 + o] = W12[o, :] * fc4_w[0, u]
    b1 = np.ascontiguousarray(bias1)                        # [120, 5] cols u
    return (a[0].astype(BF), a[1].astype(BF), w11t.astype(BF),
            m2.astype(BF), b1.astype(np.float32))


def kernel(x, W11, fc2_w, bias1, W12, fc4_w, bias2, _trace=False):
    x = np.asarray(x, dtype=np.float32)
    W11 = np.asarray(W11, np.float32)
    fc2_w = np.asarray(fc2_w, np.float32)
    bias1 = np.asarray(bias1, np.float32)
    W12 = np.asarray(W12, np.float32)
    fc4_w = np.asarray(fc4_w, np.float32)
    b2v = np.asarray(bias2, np.float32)[:, 0]

    a0, a1, w11t, m2, b1 = _host_prep(W11, fc2_w, bias1, W12, fc4_w)

    if "nc" not in _CACHE:
        _CACHE["nc"] = _build_nc()
    nc = _CACHE["nc"]

    in_maps = []
    for c in range(NCORES):
        xs = x[c * BC : (c + 1) * BC]
        xtc = xs.transpose(1, 2, 0).reshape(KF, BC).astype(BF)
        in_maps.append({"xt": xtc, "a0": a0, "a1": a1, "w11t": w11t,
                        "m2": m2, "b1": b1})

    res = run_bass_kernel_spmd(nc, in_maps, core_ids=list(range(NCORES)), trace=_trace)
    outs = [np.asarray(res.results[c]["outT"], np.float32) for c in range(NCORES)]
    full = np.concatenate(outs, axis=1).T + b2v[None, :]
    if _trace:
        kernel.last_exec_time_ns = res.exec_time_ns
    return full.astype(np.float32)


# revision 10
# speedup vs baseline: 1.4983x; 1.4983x over previous
"""Trainium2 Bass kernel for nn_BL_36721970381090 (dense_mlp).

Math: the reference network collapses (einsum commutation + Kronecker
structure) to, per batch row b:
    z[d, u]   = sum_s fc2_w[u, s] * x[b, d, s]          (200 feats)
    item2[t,u]= sum_d W11[t, d] * z[d, u] + bias1[t, u] (600 feats)
    out[b, o] = sum_{t,u} W12[o, t] * fc4_w[0, u] * relu(item2[t, u]) + bias2[o, 0]

Strategy: pure data parallel over 8 NeuronCores (batch 131072 -> 8 x 16384),
bf16 moving tensors / fp32 psum. Per 512-col block only 14 matmul passes:
  stage1 (4 mm): x in 4 chunks [100,512]; chunk pair p accumulates into one
     psum half pz[:,p,:] via block-diagonal stationaries whose columns are
     ordered (q, dl, u) -> z row q*50 + dl*5 + u, i.e. d-major, u-minor.
  z copy (2 engine ops) psum->sbuf bf16 [100,512] per half.
  gather (2 DMAs): the (q,dl,u) order makes  z half [100,512] -> y[20h:20h+20,
     5, 512] a pure linearization reshape, so one SBUF->SBUF DMA per half
     lands y[d, u, b] with d contiguous per u.
  stage2 (5 mm): ps1_u [120,512] = w11t [40,120].T @ y[:,u,:]   (K=40)
  relu+bias (5 engine ops, spread over ACT/DVE/GPSIMD) -> r_u [120,512] bf16
  stage3 (5 mm): ps2 [3,512] += m2_u [120,3].T @ r_u
Stage2/3 of sub-block i-1 are emitted after stage1 of sub-block i (software
pipelining) so the PE never waits on the copy->gather latency.
bias2 folded on host after gather.
"""

import numpy as np
import ml_dtypes
from contextlib import ExitStack

import concourse.bass as bass
import concourse.bacc as bacc
import concourse.mybir as mybir
from concourse.bass import ds
from concourse.tile import TileContext
from concourse.bass_utils import run_bass_kernel_spmd

B, D1, D2 = 131072, 40, 10
T0, T1, O0 = 120, 5, 3
NCORES = 8
BC = B // NCORES          # 16384 batch per core
KF = D1 * D2              # 400 input features (d, s)
KC = 100                  # stage-1 K-chunk (4 chunks of 100 partitions)
NB = 512                  # matmul free-dim block (1 PSUM bank fp32)
NBD = 2048                # DMA block (4 x NB)
NSB = BC // NB            # 32 sub-blocks per core

F32 = mybir.dt.float32
BF16 = mybir.dt.bfloat16
BF = ml_dtypes.bfloat16
RELU = mybir.ActivationFunctionType.Relu
COPY = mybir.ActivationFunctionType.Copy
ADD = mybir.AluOpType.add
MAX = mybir.AluOpType.max

_CACHE = {}


def _build_nc():
    nc = bacc.Bacc()
    xt = nc.dram_tensor("xt", (KF, BC), BF16, kind="ExternalInput")
    a0_h = nc.dram_tensor("a0", (KC, KC), BF16, kind="ExternalInput")
    a1_h = nc.dram_tensor("a1", (KC, KC), BF16, kind="ExternalInput")
    w11t_h = nc.dram_tensor("w11t", (D1, T0), BF16, kind="ExternalInput")
    m2_h = nc.dram_tensor("m2", (T0, T1 * O0), BF16, kind="ExternalInput")
    b1_h = nc.dram_tensor("b1", (T0, T1), F32, kind="ExternalInput")
    outT = nc.dram_tensor("outT", (O0, BC), F32, kind="ExternalOutput")

    with TileContext(nc) as tc, ExitStack() as ctx:
        consts = ctx.enter_context(tc.tile_pool(name="consts", bufs=1))
        a_sb = [consts.tile([KC, KC], BF16, tag=f"a{q}", name=f"a{q}") for q in range(2)]
        nc.sync.dma_start(a_sb[0][:, :], a0_h[:, :])
        nc.sync.dma_start(a_sb[1][:, :], a1_h[:, :])
        w11t_sb = consts.tile([D1, T0], BF16, tag="w11t")
        nc.sync.dma_start(w11t_sb[:, :], w11t_h[:, :])
        m2_sb = consts.tile([T0, T1 * O0], BF16, tag="m2")
        nc.sync.dma_start(m2_sb[:, :], m2_h[:, :])
        b1_sb = consts.tile([T0, T1], F32, tag="b1")
        nc.sync.dma_start(b1_sb[:, :], b1_h[:, :])

        xpool = ctx.enter_context(tc.tile_pool(name="xp", bufs=3))
        zpool = ctx.enter_context(tc.tile_pool(name="zp", bufs=2))
        ypool = ctx.enter_context(tc.tile_pool(name="yp", bufs=3))
        rpool = ctx.enter_context(tc.tile_pool(name="rp", bufs=2))
        opool = ctx.enter_context(tc.tile_pool(name="op", bufs=2))
        pzp = ctx.enter_context(tc.tile_pool(name="pz", bufs=1, space="PSUM"))
        ps1p = ctx.enter_context(tc.tile_pool(name="ps1", bufs=4, space="PSUM"))
        ps2p = ctx.enter_context(tc.tile_pool(name="ps2", bufs=2, space="PSUM"))

        xtiles = {}          # sub-block -> list of 4 moving APs
        osb_map = {}         # blk -> osb tile
        fetched = set()

        def xfetch(blk):
            if blk in fetched or blk >= NSB // (NBD // NB):
                return
            fetched.add(blk)
            if blk == 0:
                for jj2 in range(NBD // NB):
                    tl = [xpool.tile([KC, NB], BF16, tag=f"w{k}_{jj2}",
                                     name=f"x0_{k}_{jj2}") for k in range(4)]
                    for k in range(4):
                        nc.sync.dma_start(tl[k][:, :],
                                          xt[ds(k * KC, KC), ds(jj2 * NB, NB)])
                    xtiles[jj2] = [t[:, :] for t in tl]
            else:
                tl = [xpool.tile([KC, NBD], BF16, tag=f"x{k}", name=f"xk{k}_{blk}")
                      for k in range(4)]
                for k in range(4):
                    nc.sync.dma_start(tl[k][:, :],
                                      xt[ds(k * KC, KC), ds(blk * NBD, NBD)])
                for jj2 in range(NBD // NB):
                    xtiles[blk * 4 + jj2] = [t[:, ds(jj2 * NB, NB)] for t in tl]

        def stage1(i):
            blk, jj = divmod(i, NBD // NB)
            xfetch(blk)
            if jj == 1:
                xfetch(blk + 1)
            xs = xtiles.pop(i)
            pz = pzp.tile([KC, 2, NB], F32, tag="pz", name=f"pz{i}")
            for p in range(2):
                nc.tensor.matmul(pz[:, p, :], a_sb[0][:, :], xs[2 * p],
                                 start=True, stop=False)
                nc.tensor.matmul(pz[:, p, :], a_sb[1][:, :], xs[2 * p + 1],
                                 start=False, stop=True)
            z0 = zpool.tile([KC, NB], BF16, tag="z0", name=f"z0_{i}")
            z1 = zpool.tile([KC, NB], BF16, tag="z1", name=f"z1_{i}")
            nc.scalar.activation(z0[:, :], pz[:, 0, :], COPY)
            nc.vector.tensor_copy(z1[:, :], pz[:, 1, :])
            y = ypool.tile([D1, T1, NB], BF16, tag="y", name=f"y{i}")
            nc.sync.dma_start(y[ds(0, 20), :, :], z0[:, :])
            nc.sync.dma_start(y[ds(20, 20), :, :], z1[:, :])
            return y

        def stage23(i, y):
            blk, jj = divmod(i, NBD // NB)
            if jj == 0:
                osb_map[blk] = opool.tile([O0, NBD], F32, tag="osb", name=f"osb{blk}")
            osb = osb_map[blk]
            rtiles = []
            for u in range(T1):
                pp = ps1p.tile([T0, NB], F32, tag="ps1", name=f"pp{u}_{i}")
                nc.tensor.matmul(pp[:, :], w11t_sb[:, :], y[:, u, :],
                                 start=True, stop=True)
                r = rpool.tile([T0, NB], BF16, tag=f"r{u}", name=f"rt{u}_{i}")
                if u < 3:
                    nc.scalar.activation(r[:, :], pp[:, :], RELU,
                                         bias=b1_sb[:, ds(u, 1)])
                else:
                    nc.vector.tensor_scalar(r[:, :], pp[:, :],
                                            b1_sb[:, ds(u, 1)], 0.0,
                                            op0=ADD, op1=MAX)
                rtiles.append(r)
            ps2 = ps2p.tile([O0, NB], F32, tag="ps2", name=f"ps2_{i}")
            for u in range(T1):
                nc.tensor.matmul(ps2[:, :], m2_sb[:, ds(u * O0, O0)],
                                 rtiles[u][:, :],
                                 start=(u == 0), stop=(u == T1 - 1))
            nc.vector.tensor_copy(osb[:, ds(jj * NB, NB)], ps2[:, :])
            if jj == NBD // NB - 1:
                nc.sync.dma_start(outT[:, ds(blk * NBD, NBD)], osb[:, :])

        ys = {}
        for i in range(NSB):
            ys[i] = stage1(i)
            if i >= 2:
                stage23(i - 2, ys.pop(i - 2))
        stage23(NSB - 2, ys.pop(NSB - 2))
        stage23(NSB - 1, ys.pop(NSB - 1))
    nc.finalize()
    return nc


def _host_prep(W11, fc2_w, bias1, W12, fc4_w):
    # stage-1 stationaries: row rc = dl*10 + s; col m = q*50 + dl*5 + u
    a = np.zeros((2, KC, KC), np.float32)
    for q in range(2):
        for dl in range(10):
            for s in range(10):
                for u in range(T1):
                    a[q, dl * 10 + s, q * 50 + dl * 5 + u] = fc2_w[u, s]
    w11t = np.ascontiguousarray(W11.T)                      # [40, 120]
    m2 = np.empty((T0, T1 * O0), np.float32)                # [t, u*3+o]
    for u in range(T1):
        for o in range(O0):
            m2[:, u * O0 + o] = W12[o, :] * fc4_w[0, u]
    b1 = np.ascontiguousarray(bias1)                        # [120, 5] cols u
    return (a[0].astype(BF), a[1].astype(BF), w11t.astype(BF),
            m2.astype(BF), b1.astype(np.float32))


def kernel(x, W11, fc2_w, bias1, W12, fc4_w, bias2, _trace=False):
    x = np.asarray(x, dtype=np.float32)
    W11 = np.asarray(W11, np.float32)
    fc2_w = np.asarray(fc2_w, np.float32)
    bias1 = np.asarray(bias1, np.float32)
    W12 = np.asarray(W12, np.float32)
    fc4_w = np.asarray(fc4_w, np.float32)
    b2v = np.asarray(bias2, np.float32)[:, 0]

    a0, a1, w11t, m2, b1 = _host_prep(W11, fc2_w, bias1, W12, fc4_w)

    if "nc" not in _CACHE:
        _CACHE["nc"] = _build_nc()
    nc = _CACHE["nc"]

    in_maps = []
    for c in range(NCORES):
        xs = x[c * BC : (c + 1) * BC]
        xtc = xs.transpose(1, 2, 0).reshape(KF, BC).astype(BF)
        in_maps.append({"xt": xtc, "a0": a0, "a1": a1, "w11t": w11t,
                        "m2": m2, "b1": b1})

    res = run_bass_kernel_spmd(nc, in_maps, core_ids=list(range(NCORES)), trace=_trace)
    outs = [np.asarray(res.results[c]["outT"], np.float32) for c in range(NCORES)]
    full = np.concatenate(outs, axis=1).T + b2v[None, :]
    if _trace:
        kernel.last_exec_time_ns = res.exec_time_ns
    return full.astype(np.float32)


# revision 17
# speedup vs baseline: 1.6016x; 1.0690x over previous
"""Trainium2 Bass kernel for nn_BL_36721970381090 (dense_mlp).

Math: the reference network collapses (einsum commutation + Kronecker
structure) to, per batch row b:
    z[d, u]   = sum_s fc2_w[u, s] * x[b, d, s]          (200 feats)
    item2[t,u]= sum_d W11[t, d] * z[d, u] + bias1[t, u] (600 feats)
    out[b, o] = sum_{t,u} W12[o, t] * fc4_w[0, u] * relu(item2[t, u]) + bias2[o, 0]

Strategy: pure data parallel over 8 NeuronCores (batch 131072 -> 8 x 16384),
bf16 moving tensors / fp32 psum. Per 512-col sub-block only 14 matmul passes
(vs 19 naive) and only 5 PSUM-draining engine ops (vs 8):
  stage1 (4 mm): x in 4 chunks [100,512]; chunk pair p accumulates into psum
     half pz[:,p,:] via block-diagonal stationaries whose columns are ordered
     (q, dl, u) -> z row q*50 + dl*5 + u (d-major, u-minor).
  z copy (1 op, [100,1024]) psum->sbuf bf16.
  gather (2 DMAs): the (q,dl,u) order makes z half [100,512] -> y[20h:20h+20,
     5,512] a pure linearization reshape (SBUF->SBUF DMA partition fold).
  stage2 (5 mm): ps1 [120,512] = w11b_u [41,120].T @ y[0:41,u,:]  (K=41);
     y row 40 holds ones (gpsimd memset) and w11b row 40 holds bias1[:,u],
     so bias1 is added by the matmul itself.
  relu (3 ops: [120,1024] x2 pair-merged + [120,512]) -> r bf16, no bias.
  stage3 (5 mm): ps2 [3,512] += m2_u [120,3].T @ r_u.
Stage2/3 of sub-block i-2 are emitted after stage1 of sub-block i (2-deep
software pipelining) so the PE never waits on the copy->gather latency and
stays DVFS-ramped. bias2 folded on host after gather.
"""

import numpy as np
import ml_dtypes
from contextlib import ExitStack

import concourse.bass as bass
import concourse.bacc as bacc
import concourse.mybir as mybir
from concourse.bass import ds
from concourse.tile import TileContext
from concourse.bass_utils import run_bass_kernel_spmd

B, D1, D2 = 131072, 40, 10
T0, T1, O0 = 120, 5, 3
NCORES = 8
BC = B // NCORES          # 16384 batch per core
KF = D1 * D2              # 400 input features (d, s)
KC = 100                  # stage-1 K-chunk (4 chunks of 100 partitions)
NB = 512                  # matmul free-dim block (1 PSUM bank fp32)
NBD = 2048                # DMA block (4 x NB)
NSB = BC // NB            # 32 sub-blocks per core

F32 = mybir.dt.float32
BF16 = mybir.dt.bfloat16
BF = ml_dtypes.bfloat16
RELU = mybir.ActivationFunctionType.Relu
COPY = mybir.ActivationFunctionType.Copy
ADD = mybir.AluOpType.add
MAX = mybir.AluOpType.max

_CACHE = {}


def _build_nc():
    nc = bacc.Bacc()
    xt = nc.dram_tensor("xt", (KF, BC), BF16, kind="ExternalInput")
    a0_h = nc.dram_tensor("a0", (KC, KC), BF16, kind="ExternalInput")
    a1_h = nc.dram_tensor("a1", (KC, KC), BF16, kind="ExternalInput")
    w11b_h = nc.dram_tensor("w11b", (D1 + 1, T1 * T0), BF16, kind="ExternalInput")
    m2_h = nc.dram_tensor("m2", (T0, T1 * O0), BF16, kind="ExternalInput")
    ones_h = nc.dram_tensor("ones", (1, T1 * NB), BF16, kind="ExternalInput")
    outT = nc.dram_tensor("outT", (O0, BC), F32, kind="ExternalOutput")

    with TileContext(nc) as tc, ExitStack() as ctx:
        consts = ctx.enter_context(tc.tile_pool(name="consts", bufs=1))
        a_sb = [consts.tile([KC, KC], BF16, tag=f"a{q}", name=f"a{q}") for q in range(2)]
        nc.sync.dma_start(a_sb[0][:, :], a0_h[:, :])
        nc.sync.dma_start(a_sb[1][:, :], a1_h[:, :])
        w11b_sb = consts.tile([D1 + 1, T1 * T0], BF16, tag="w11b")
        nc.sync.dma_start(w11b_sb[:, :], w11b_h[:, :])
        m2_sb = consts.tile([T0, T1 * O0], BF16, tag="m2")
        nc.sync.dma_start(m2_sb[:, :], m2_h[:, :])

        xpool = ctx.enter_context(tc.tile_pool(name="xp", bufs=3))
        zpool = ctx.enter_context(tc.tile_pool(name="zp", bufs=2))
        ypool = ctx.enter_context(tc.tile_pool(name="yp", bufs=3))
        rpool = ctx.enter_context(tc.tile_pool(name="rp", bufs=2))
        opool = ctx.enter_context(tc.tile_pool(name="op", bufs=2))
        pzp = ctx.enter_context(tc.tile_pool(name="pz", bufs=1, space="PSUM"))
        ps1p = ctx.enter_context(tc.tile_pool(name="ps1", bufs=1, space="PSUM"))
        ps2p = ctx.enter_context(tc.tile_pool(name="ps2", bufs=1, space="PSUM"))

        # pre-warm the ones row (partition 40) of each y buffer once
        for w in range(3):
            yw = ypool.tile([D1 + 1, T1, NB], BF16, tag="y", name=f"ywarm{w}")
            nc.sync.dma_start(yw[ds(D1, 1), :, :], ones_h[:, :])

        xtiles = {}
        osb_map = {}
        fetched = set()

        def xfetch(blk):
            if blk in fetched or blk >= NSB // (NBD // NB):
                return
            fetched.add(blk)
            if blk == 0:
                for jj2 in range(NBD // NB):
                    tl = [xpool.tile([KC, NB], BF16, tag=f"w{k}_{jj2}",
                                     name=f"x0_{k}_{jj2}") for k in range(4)]
                    for k in range(4):
                        nc.sync.dma_start(tl[k][:, :],
                                          xt[ds(k * KC, KC), ds(jj2 * NB, NB)])
                    xtiles[jj2] = [t[:, :] for t in tl]
            else:
                tl = [xpool.tile([KC, NBD], BF16, tag=f"x{k}", name=f"xk{k}_{blk}")
                      for k in range(4)]
                for k in range(4):
                    nc.sync.dma_start(tl[k][:, :],
                                      xt[ds(k * KC, KC), ds(blk * NBD, NBD)])
                for jj2 in range(NBD // NB):
                    xtiles[blk * 4 + jj2] = [t[:, ds(jj2 * NB, NB)] for t in tl]

        def stage1(i):
            blk, jj = divmod(i, NBD // NB)
            xfetch(blk)
            if jj == 1:
                xfetch(blk + 1)
            xs = xtiles.pop(i)
            pz = pzp.tile([KC, 2, NB], F32, tag="pz", name=f"pz{i}")
            for p in range(2):
                nc.tensor.matmul(pz[:, p, :], a_sb[0][:, :], xs[2 * p],
                                 start=True, stop=False)
                nc.tensor.matmul(pz[:, p, :], a_sb[1][:, :], xs[2 * p + 1],
                                 start=False, stop=True)
            z = zpool.tile([KC, 2, NB], BF16, tag="z", name=f"z{i}")
            nc.scalar.activation(z[:, :, :], pz[:, :, :], COPY)
            y = ypool.tile([D1 + 1, T1, NB], BF16, tag="y", name=f"y{i}")
            nc.sync.dma_start(y[ds(0, 20), :, :], z[:, 0, :])
            nc.sync.dma_start(y[ds(20, 20), :, :], z[:, 1, :])
            return y

        def stage23(i, y):
            blk, jj = divmod(i, NBD // NB)
            if jj == 0:
                osb_map[blk] = opool.tile([O0, NBD], F32, tag="osb", name=f"osb{blk}")
            osb = osb_map[blk]
            ppab = ps1p.tile([T0, 2, NB], F32, tag="ps1a", name=f"ppab{i}")
            ppcd = ps1p.tile([T0, 2, NB], F32, tag="ps1b", name=f"ppcd{i}")
            ppe = ps1p.tile([T0, NB], F32, tag="ps1c", name=f"ppe{i}")
            pslot = [ppab[:, 0, :], ppab[:, 1, :], ppcd[:, 0, :], ppcd[:, 1, :],
                     ppe[:, :]]
            for u in range(T1):
                nc.tensor.matmul(pslot[u], w11b_sb[:, ds(u * T0, T0)],
                                 y[:, u, :], start=True, stop=True)
            rab = rpool.tile([T0, 2, NB], BF16, tag="rab", name=f"rab{i}")
            rcd = rpool.tile([T0, 2, NB], BF16, tag="rcd", name=f"rcd{i}")
            re = rpool.tile([T0, NB], BF16, tag="re", name=f"re{i}")
            nc.scalar.activation(rab[:, :, :], ppab[:, :, :], RELU)
            nc.vector.tensor_scalar_max(rcd[:, :, :], ppcd[:, :, :], 0.0)
            nc.vector.tensor_scalar_max(re[:, :], ppe[:, :], 0.0)
            rslot = [rab[:, 0, :], rab[:, 1, :], rcd[:, 0, :], rcd[:, 1, :],
                     re[:, :]]
            ps2 = ps2p.tile([O0, NB], F32, tag="ps2", name=f"ps2_{i}")
            for u in range(T1):
                nc.tensor.matmul(ps2[:, :], m2_sb[:, ds(u * O0, O0)],
                                 rslot[u], start=(u == 0), stop=(u == T1 - 1))
            nc.vector.tensor_copy(osb[:, ds(jj * NB, NB)], ps2[:, :])
            if jj == NBD // NB - 1:
                nc.sync.dma_start(outT[:, ds(blk * NBD, NBD)], osb[:, :])

        ys = {}
        for i in range(NSB):
            ys[i] = stage1(i)
            if i >= 2:
                stage23(i - 2, ys.pop(i - 2))
        stage23(NSB - 2, ys.pop(NSB - 2))
        stage23(NSB - 1, ys.pop(NSB - 1))
    nc.finalize()
    return nc


def _host_prep(W11, fc2_w, bias1, W12, fc4_w):
    # stage-1 stationaries: row rc = dl*10 + s; col m = q*50 + dl*5 + u
    a = np.zeros((2, KC, KC), np.float32)
    for q in range(2):
        for dl in range(10):
            for s in range(10):
                for u in range(T1):
                    a[q, dl * 10 + s, q * 50 + dl * 5 + u] = fc2_w[u, s]
    # stage-2 stationary with bias row: [41, u*120 + t]
    w11b = np.zeros((D1 + 1, T1 * T0), np.float32)
    for u in range(T1):
        w11b[:D1, u * T0 : (u + 1) * T0] = W11.T
        w11b[D1, u * T0 : (u + 1) * T0] = bias1[:, u]
    m2 = np.empty((T0, T1 * O0), np.float32)                # [t, u*3+o]
    for u in range(T1):
        for o in range(O0):
            m2[:, u * O0 + o] = W12[o, :] * fc4_w[0, u]
    return (a[0].astype(BF), a[1].astype(BF), w11b.astype(BF), m2.astype(BF))


def kernel(x, W11, fc2_w, bias1, W12, fc4_w, bias2, _trace=False):
    x = np.asarray(x, dtype=np.float32)
    W11 = np.asarray(W11, np.float32)
    fc2_w = np.asarray(fc2_w, np.float32)
    bias1 = np.asarray(bias1, np.float32)
    W12 = np.asarray(W12, np.float32)
    fc4_w = np.asarray(fc4_w, np.float32)
    b2v = np.asarray(bias2, np.float32)[:, 0]

    a0, a1, w11b, m2 = _host_prep(W11, fc2_w, bias1, W12, fc4_w)
    ones = np.ones((1, T1 * NB), np.float32).astype(BF)

    if "nc" not in _CACHE:
        _CACHE["nc"] = _build_nc()
    nc = _CACHE["nc"]

    in_maps = []
    for c in range(NCORES):
        xs = x[c * BC : (c + 1) * BC]
        xtc = xs.transpose(1, 2, 0).reshape(KF, BC).astype(BF)
        in_maps.append({"xt": xtc, "a0": a0, "a1": a1, "w11b": w11b, "m2": m2,
                        "ones": ones})

    res = run_bass_kernel_spmd(nc, in_maps, core_ids=list(range(NCORES)), trace=_trace)
    outs = [np.asarray(res.results[c]["outT"], np.float32) for c in range(NCORES)]
    full = np.concatenate(outs, axis=1).T + b2v[None, :]
    if _trace:
        kernel.last_exec_time_ns = res.exec_time_ns
    return full.astype(np.float32)


# revision 18
# speedup vs baseline: 2.0667x; 1.2904x over previous
"""Trainium2 Bass kernel for nn_BL_36721970381090 (dense_mlp).

Math: the reference network
    item1 = einsum("td,bds->bts", W11, x)
    item2 = relu(einsum("bts,us->btu", item1, fc2_w) + bias1)
    item3 = einsum("ot,btu->bou", W12, item2)
    out   = (einsum("bou,pu->bop", item3, fc4_w) + bias2)[..., 0]
collapses (Kronecker identity) to a plain 2-layer MLP applied per batch row:
    out[b] = M2 @ relu(M1 @ vec(x[b]) + b1) + b2
with M1 = kron(W11, fc2_w) [600, 400], M2 = kron(W12, fc4_w) [3, 600],
b1 = bias1.reshape(600), b2 = bias2[:, 0].

Strategy: pure data parallel over 8 NeuronCores (batch split 131072 -> 8 x
16384). Host pre-transposes x to feature-major xT [400, Bc] per core and casts
to bf16 (input is the only large tensor; bf16 halves HBM traffic and doubles
PE throughput vs fp32's 2-pass matmul). On-chip: feature-major pipeline with
batch in the moving free dim - no on-chip transposes at all.
  layer1: psum[(t,u) chunk 120, b 512] += M1T_k[100,120].T @ xT_k[100,512]
          (4 K-chunks x 5 M-chunks)
  relu+bias1 on ScalarE (PSUM -> SBUF, cast to bf16)
  layer2: psum[3, b 512] += M2T_m[120,3].T @ relu_m[120,512]  (5 chunks)
  bias2 folded on host after gather.
"""

import numpy as np
import ml_dtypes
from contextlib import ExitStack

import concourse.bass as bass
import concourse.bacc as bacc
import concourse.mybir as mybir
from concourse.bass import ds
from concourse.tile import TileContext
from concourse.bass_utils import run_bass_kernel_spmd

B, D1, D2 = 131072, 40, 10
T0, T1, O0 = 120, 5, 3
NCORES = 8
BC = B // NCORES          # 16384 batch per core
KF = D1 * D2              # 400 input features (d, s)
MF = T0 * T1              # 600 hidden features (t, u)
KC = 100                  # K-chunk (4 chunks of 100 partitions)
MC = 120                  # M-chunk (5 chunks of 120 partitions)
NB = 512                  # matmul free-dim block (1 PSUM bank fp32)
NBD = 2048                # DMA block (4 x NB)

F32 = mybir.dt.float32
BF16 = mybir.dt.bfloat16
BF = ml_dtypes.bfloat16
RELU = mybir.ActivationFunctionType.Relu
COPY = mybir.ActivationFunctionType.Copy
ADD = mybir.AluOpType.add
MAX = mybir.AluOpType.max

_CACHE = {}


def _build_nc():
    nc = bacc.Bacc()
    xt = nc.dram_tensor("xt", (KF, BC), BF16, kind="ExternalInput")
    ablk = nc.dram_tensor("ablk", (KC, 64), BF16, kind="ExternalInput")
    l2a = nc.dram_tensor("l2a", (128, MF), BF16, kind="ExternalInput")
    l2b = nc.dram_tensor("l2b", (128, MF), BF16, kind="ExternalInput")
    m2t = nc.dram_tensor("m2t", (MC, 5 * O0), BF16, kind="ExternalInput")
    b1 = nc.dram_tensor("b1", (MC, 5), F32, kind="ExternalInput")
    outT = nc.dram_tensor("outT", (O0, BC), F32, kind="ExternalOutput")

    nm = MF // MC  # 5

    with TileContext(nc) as tc, ExitStack() as ctx:
        consts = ctx.enter_context(tc.tile_pool(name="consts", bufs=1))
        a_sb = consts.tile([KC, 64], BF16, tag="a")
        nc.sync.dma_start(a_sb[:, :], ablk[:, :])
        l2_sb = [consts.tile([128, MF], BF16, tag=f"l2_{p}", name=f"l2sb{p}") for p in range(2)]
        nc.sync.dma_start(l2_sb[0][:, :], l2a[:, :])
        nc.sync.dma_start(l2_sb[1][:, :], l2b[:, :])
        m2_sb = consts.tile([MC, nm * O0], BF16, tag="m2")
        nc.sync.dma_start(m2_sb[:, :], m2t[:, :])
        b1_sb = consts.tile([MC, nm], F32, tag="b1")
        nc.sync.dma_start(b1_sb[:, :], b1[:, :])
        xpool = ctx.enter_context(tc.tile_pool(name="xp", bufs=3))
        opool = ctx.enter_context(tc.tile_pool(name="op", bufs=2))
        zpool = ctx.enter_context(tc.tile_pool(name="zp", bufs=3))
        rpool = ctx.enter_context(tc.tile_pool(name="rp", bufs=6))
        pzp = ctx.enter_context(tc.tile_pool(name="pz", bufs=2, space="PSUM"))
        ps1p = ctx.enter_context(tc.tile_pool(name="ps1", bufs=4, space="PSUM"))
        ps2p = ctx.enter_context(tc.tile_pool(name="ps2", bufs=2, space="PSUM"))

        for blk in range(BC // NBD):
            if blk == 0:
                x0 = [
                    [xpool.tile([KC, NB], BF16, tag=f"w{k}_{jj}", name=f"x0_{k}_{jj}")
                     for k in range(4)]
                    for jj in range(NBD // NB)
                ]
                for jj in range(NBD // NB):
                    for k in range(4):
                        nc.sync.dma_start(
                            x0[jj][k][:, :],
                            xt[ds(k * KC, KC), ds(jj * NB, NB)],
                        )
            else:
                xk = [xpool.tile([KC, NBD], BF16, tag=f"x{k}", name=f"xk{k}") for k in range(4)]
                for k in range(4):
                    nc.sync.dma_start(xk[k][:, :], xt[ds(k * KC, KC), ds(blk * NBD, NBD)])
            for jj in range(NBD // NB):
                def xs(k):
                    return (x0[jj][k][:, :] if blk == 0
                            else xk[k][:, ds(jj * NB, NB)])
                # stage 1: z pair tiles [128, 512]; chunk 2p -> rows 0:64,
                # chunk 2p+1 -> rows 64:128 (zero-padded cols 50-63 in ablk)
                ztiles = []
                for p in range(2):
                    pz = pzp.tile([128, NB], F32, tag="pz", name=f"pz{p}{jj}")
                    nc.tensor.matmul(pz[0:64, :], a_sb[:, :], xs(2 * p),
                                     start=True, stop=True, tile_position=(0, 0))
                    nc.tensor.matmul(pz[64:128, :], a_sb[:, :], xs(2 * p + 1),
                                     start=True, stop=True, tile_position=(0, 64))
                    z = zpool.tile([128, NB], BF16, tag=f"z{p}", name=f"zt{p}{jj}")
                    if p == 0:
                        nc.scalar.activation(z[:, :], pz[:, :], COPY)
                    else:
                        nc.vector.tensor_copy(z[:, :], pz[:, :])
                    ztiles.append(z)
                # stage 2 + relu + layer 2
                rtiles = []
                for m in range(nm):
                    pp = ps1p.tile([MC, NB], F32, tag="ps1", name=f"pp{m}{jj}")
                    for p in range(2):
                        nc.tensor.matmul(
                            pp[:, :], l2_sb[p][:, ds(m * MC, MC)], ztiles[p][:, :],
                            start=(p == 0), stop=(p == 1),
                        )
                    r = rpool.tile([MC, NB], BF16, tag=f"r{m}", name=f"rt{m}{jj}")
                    if m < 3:
                        nc.scalar.activation(r[:, :], pp[:, :], RELU,
                                             bias=b1_sb[:, ds(m, 1)])
                    else:
                        nc.vector.tensor_scalar(r[:, :], pp[:, :],
                                                b1_sb[:, ds(m, 1)], 0.0,
                                                op0=ADD, op1=MAX)
                    rtiles.append(r)
                if jj == 0:
                    osb = opool.tile([O0, NBD], F32, tag="osb")
                ps2 = ps2p.tile([O0, NB], F32, tag="ps2", name=f"ps2{jj}")
                for m in range(nm):
                    nc.tensor.matmul(
                        ps2[:, :], m2_sb[:, ds(m * O0, O0)], rtiles[m][:, :],
                        start=(m == 0), stop=(m == nm - 1),
                    )
                nc.vector.tensor_copy(osb[:, ds(jj * NB, NB)], ps2[:, :])
            nc.sync.dma_start(outT[:, ds(blk * NBD, NBD)], osb[:, :])
    nc.finalize()
    return nc


def kernel(x, W11, fc2_w, bias1, W12, fc4_w, bias2, _trace=False):
    x = np.asarray(x, dtype=np.float32)
    W11 = np.asarray(W11, np.float32)
    fc2_w = np.asarray(fc2_w, np.float32)
    M2 = np.kron(np.asarray(W12, np.float32), np.asarray(fc4_w, np.float32))
    b1v = np.ascontiguousarray(np.asarray(bias1, np.float32).reshape(5, MC).T)
    b2v = np.asarray(bias2, np.float32)[:, 0]

    # stage-1 stationary: [(dl,s) 100, (u,dl') 50 + 14 zero pad]
    A = np.zeros((KC, 64), np.float32)
    A[:, :50] = np.einsum("us,de->dsue", fc2_w, np.eye(10, dtype=np.float32)).reshape(KC, 50)
    ablk = np.ascontiguousarray(A).astype(BF)
    # stage-2 stationaries: pair p rows = chunk 2p (0:50), pad, chunk 2p+1 (64:114), pad
    l2 = np.zeros((2, 128, MF), np.float32)
    for p in range(2):
        for r in range(128):
            if r < 50:
                u, dl, d = r // 10, r % 10, 10 * (2 * p) + (r % 10)
            elif 64 <= r < 114:
                u, dl, d = (r - 64) // 10, (r - 64) % 10, 10 * (2 * p + 1) + ((r - 64) % 10)
            else:
                continue
            for t in range(T0):
                l2[p, r, t * 5 + u] = W11[t, d]
    l2a = np.ascontiguousarray(l2[0]).astype(BF)
    l2b = np.ascontiguousarray(l2[1]).astype(BF)
    m2t = np.ascontiguousarray(
        M2.T.reshape(5, MC, O0).transpose(1, 0, 2).reshape(MC, 5 * O0)
    ).astype(BF)

    if "nc" not in _CACHE:
        _CACHE["nc"] = _build_nc()
    nc = _CACHE["nc"]

    in_maps = []
    for c in range(NCORES):
        xs = x[c * BC : (c + 1) * BC]
        xtc = xs.transpose(1, 2, 0).reshape(KF, BC).astype(BF)
        in_maps.append({"xt": xtc, "ablk": ablk, "l2a": l2a, "l2b": l2b,
                        "m2t": m2t, "b1": b1v})

    res = run_bass_kernel_spmd(nc, in_maps, core_ids=list(range(NCORES)), trace=_trace)
    outs = [np.asarray(res.results[c]["outT"], np.float32) for c in range(NCORES)]
    full = np.concatenate(outs, axis=1).T + b2v[None, :]
    if _trace:
        kernel.last_exec_time_ns = res.exec_time_ns
    return full.astype(np.float32)

